# revision 47
# baseline (speedup 1.0000x reference)
"""Trainium2 Bass kernel for nn_Attention_13297218748956.

Multi-head causal self-attention with RoPE (B=64, T=128, C=2048, H=16, hd=128),
data-parallel over batch across 8 NeuronCores (8 batches/core, no collectives).

Design (vs f32r baseline at ~667 us; this version ~514 us at full clock):
  - every matmul operand bf16 (1 cycle/row, LDWEIGHTS hidden in the
    background weight buffer, unlike f32r); fp32 PSUM accumulation.
    Validated numerically: rel err ~4e-3 vs the 2e-2 gate.
  - weights + xT shipped bf16 from host (halves DMA), V and per-head
    attention outputs stay SBUF-resident (no DRAM staging roundtrips).
  - attention math bf16: scores/transpose/probs@V are 128-cycle matmuls
    instead of fp32's 512.
  - softmax without max-subtraction (scores bounded ~|4.5|): exp+accum on
    ACT, one DVE mask-add, one DVE scale; rope tmps on DVE (gpsimd has no
    PSUM port and is slow).
  - prologue emitted k-outer over 6 V accs + head-0 q accs so the PE paces
    with the chunk DMA arrivals instead of stalling on whole tensors; a
    short dummy-matmul warm-up absorbs the cold p-state ramp during the
    first transfers' latency.
  - DMA descriptors each run on one hw queue (~45 GB/s): weight-stream
    prefetches (wvs/wos, 1 m-set ahead) are split 4-way for parallel
    queues; per-head wq/wk prefetched 1 head ahead on alternating rings;
    wos shares the wvs pool (wvs dead by the time wo streams in).
  - matmul operands always single-sliced 2D-tile APs (double-sliced 3D
    APs measured ~+40ns per matmul).

Per-core layout:
  xT3 [128, KC, 1024] bf16 chunk-major (1024 = 8 batches x 128 tokens)
  wq4/wk4 [H,128,KC,128] bf16: head col-blocks, partitions = contraction
  wv4/wo4 [4,128,KC,512] bf16: 512-wide m-set blocks
  phase 1 (per m): V m-set -> v[m] SBUF [128, 8, 512] bf16 (partition=token)
  phase 2 (per head): QT/KT = wcol.T @ xT -> rope in [d,t] layout (pair-swap
    matmul + cos/sin elementwise, 1/sqrt(hd) folded into q's cos/sin) ->
    per batch: scores -> mask-add -> exp -> scale -> PE-transpose ->
    attnT[n][:, b] = V.T @ probsT, attnT resident [128,1024] bf16 x16.
  phase 3: y = attnT.T @ woT streaming from SBUF, fp32 out.
"""

import numpy as np
import ml_dtypes

import concourse.bacc as bacc
import concourse.tile as tile
import concourse.mybir as mybir
from concourse.bass_utils import run_bass_kernel_spmd

N_CORES = 8
B, T, C, H = 64, 128, 2048, 16
HD = C // H  # 128
BPC = B // N_CORES  # 8 batches per core
TOK = BPC * T  # 1024 tokens per core
KC = C // 128  # 16 contraction chunks
TH = TOK // 512  # 2 token halves for 512-wide moving operand
F32 = mybir.dt.float32
BF16 = mybir.dt.bfloat16
BF = ml_dtypes.bfloat16

_CACHE = {}


def _build():
    nc = bacc.Bacc("TRN2", target_bir_lowering=False, debug=False)

    # xT3 is chunk-major: [p, k, t] = xT[k*128+p, t], so one SBUF tile can
    # receive the whole tensor in a few large descriptors
    xT3 = nc.dram_tensor("xT3", [128, KC, TOK], BF16, kind="ExternalInput")
    wq4 = nc.dram_tensor("wq4", [H, 128, KC, 128], BF16, kind="ExternalInput")
    wk4 = nc.dram_tensor("wk4", [H, 128, KC, 128], BF16, kind="ExternalInput")
    wv4 = nc.dram_tensor("wv4", [4, 128, KC, 512], BF16, kind="ExternalInput")
    wo4 = nc.dram_tensor("wo4", [4, 128, KC, 512], BF16, kind="ExternalInput")
    # stid = st | id packed; ropec = cosq|sinq|cosk|sink packed
    stid = nc.dram_tensor("stid", [128, 256], BF16, kind="ExternalInput")
    maskd = nc.dram_tensor("maskd", [128, 128], F32, kind="ExternalInput")
    ropec = nc.dram_tensor("ropec", [128, 2048], BF16, kind="ExternalInput")
    y = nc.dram_tensor("y", [TOK, C], F32, kind="ExternalOutput")

    with tile.TileContext(nc) as tc:
        with (
            tc.tile_pool(name="consts", bufs=1) as consts,
            tc.tile_pool(name="xt", bufs=1) as xtp,
            tc.tile_pool(name="vres", bufs=2) as vp,
            tc.tile_pool(name="attnres", bufs=1) as attnp,
            tc.tile_pool(name="wstream", bufs=2) as wsp,
            tc.tile_pool(name="wcol", bufs=4) as wcolp,
            tc.tile_pool(name="ps512", bufs=4, space="PSUM") as ps512,
            tc.tile_pool(name="psc", bufs=2, space="PSUM") as psc,
            tc.tile_pool(name="mix", bufs=2, space="PSUM") as mixp,
        ):
            stid_t = consts.tile([128, 256], BF16)
            st_t = stid_t[:, 0:128]
            id_t = stid_t[:, 128:256]
            mask_t = consts.tile([128, 128], F32)
            ropec_t = consts.tile([128, 2048], BF16)
            cosq_t = ropec_t[:, 0:512]
            sinq_t = ropec_t[:, 512:1024]
            cosk_t = ropec_t[:, 1024:1536]
            sink_t = ropec_t[:, 1536:2048]

            # ---- prologue DMAs. Descriptors each run on a single hw queue
            # (~45 GB/s), so bandwidth needs MANY in flight; 2D tiles keep
            # the matmul operand APs cheap (3D-sliced operands cost ~+40ns
            # per matmul, measured) ----
            # PE warm-up: a short dummy matmul stream with no DMA deps runs
            # during the first transfers' latency, ramping the clock out of
            # the cold p-state before real work arrives
            warm = consts.tile([128, 512], BF16, name="warm")
            nc.vector.memset(warm[:], 0.0)
            wacc = ps512.tile([128, 512], F32, tag="ps512", name="wacc")
            for _ in range(8):
                nc.tensor.matmul(wacc[:], warm[:, 0:128], warm[:],
                                 start=True, stop=True)

            xt = []
            for k in range(KC):
                xk = xtp.tile([128, TOK], BF16, tag=f"xt{k}", name=f"xt{k}")
                xt.append(xk)
            # sync: xt chunks in k order, first split for a fast start
            nc.sync.dma_start(out=xt[0][:, 0:128], in_=xT3[:, 0, 0:128])
            nc.sync.dma_start(out=xt[0][:, 128:256], in_=xT3[:, 0, 128:256])
            for q in range(1, 4):
                nc.sync.dma_start(out=xt[0][:, q * 256:(q + 1) * 256],
                                  in_=xT3[:, 0, q * 256:(q + 1) * 256])
            nc.sync.dma_start(out=xt[1][:, 0:512], in_=xT3[:, 1, 0:512])
            nc.sync.dma_start(out=xt[1][:, 512:1024], in_=xT3[:, 1, 512:1024])
            for k in range(2, KC - 2):
                nc.sync.dma_start(out=xt[k][:], in_=xT3[:, k, :])

            # scalar: wv m-set 0 per-k (paces the prologue V-pass), packed
            # consts interleaved by need time
            wvs0 = wsp.tile([128, KC, 512], BF16, tag="ws", name="wvs0")
            for k in range(KC):
                if k == 0:
                    # halves so the split first matmul starts sooner
                    nc.scalar.dma_start(out=wvs0[:, 0, 0:256],
                                        in_=wv4[0, :, 0, 0:256])
                    nc.scalar.dma_start(out=wvs0[:, 0, 256:512],
                                        in_=wv4[0, :, 0, 256:512])
                else:
                    nc.scalar.dma_start(out=wvs0[:, k, :], in_=wv4[0, :, k, :])
                if k == 1:
                    nc.scalar.dma_start(out=stid_t[:], in_=stid[:])
                elif k == 3:
                    nc.scalar.dma_start(out=ropec_t[:], in_=ropec[:])
                elif k == 7:
                    nc.scalar.dma_start(out=mask_t[:], in_=maskd[:])
            # head 0/1 weights in 4-chunk groups (subtile deps let proj start
            # before the full 0.5MB lands)
            wcol = {}

            def _wcol_dma(n, kind, eng, groups=1):
                wT_ = wq4 if kind == "q" else wk4
                wc = wcolp.tile([128, KC, 128], BF16, tag="wcol",
                                name=f"wcol_{kind}{n}")
                gsz = KC // groups
                for g in range(groups):
                    eng.dma_start(out=wc[:, g * gsz:(g + 1) * gsz, :],
                                  in_=wT_[n, :, g * gsz:(g + 1) * gsz, :])
                wcol[(n, kind)] = wc
            # wq0 first and finely split: the prologue q-interleave matmuls
            # sit in the PE queue, so a late wq0 chunk stalls everything.
            # gpsimd carries ONLY the head-0/1 weights: a late wq1 caused a
            # 4us PE gap + HAM re-throttle
            _wcol_dma(0, "q", nc.gpsimd, groups=4)
            # two late xt chunks ride gpsimd's less-congested issue queue
            for k in range(KC - 2, KC):
                nc.gpsimd.dma_start(out=xt[k][:], in_=xT3[:, k, :])
            _wcol_dma(0, "k", nc.gpsimd, groups=2)
            _wcol_dma(1, "q", nc.gpsimd, groups=2)
            _wcol_dma(1, "k", nc.gpsimd, groups=2)

            v = {}
            attn = []
            for n in range(H):
                an = attnp.tile([128, TOK], BF16, tag=f"at{n}", name=f"at{n}")
                attn.append(an)

            with (
                tc.tile_pool(name="qkh", bufs=2) as qkhp,
                tc.tile_pool(name="qtstage", bufs=2) as stagep,
                tc.tile_pool(name="ropet", bufs=2) as ropep,
                tc.tile_pool(name="soft", bufs=3) as softp,
                tc.tile_pool(name="small", bufs=4) as smallp,
            ):

                def emit_vpass(m, wvs, interleave):
                    v_m = vp.tile([128, BPC, 512], BF16, tag="v", name=f"v{m}")
                    v[m] = v_m
                    if interleave:
                        # k-outer over 6 V accs + head-0 q accs (borrowing
                        # the psc slots, which grow to bank size): 8 matmuls
                        # per chunk paces the PE with the chunk DMA arrivals
                        accs = [
                            ps512.tile([128, 512], F32, tag="ps512",
                                       name=f"vacc{tt}")
                            for tt in range(4)
                        ] + [
                            mixp.tile([128, 512], F32, tag="mix",
                                      name=f"vacc{tt}")
                            for tt in range(4, 6)
                        ]
                        qacc = [
                            psc.tile([128, 512], F32, tag="sc",
                                     name=f"p0qacc{th}")
                            for th in range(TH)
                        ]
                        wq0 = wcol[(0, "q")]
                        for k in range(KC):
                            for tt in range(6):
                                nc.tensor.matmul(
                                    accs[tt][:],
                                    xt[k][:, tt * 128:(tt + 1) * 128],
                                    wvs[:, k, :],
                                    start=(k == 0), stop=(k == KC - 1),
                                )
                            for th in range(TH):
                                nc.tensor.matmul(
                                    qacc[th][:],
                                    wq0[:, k, :],
                                    xt[k][:, th * 512:(th + 1) * 512],
                                    start=(k == 0), stop=(k == KC - 1),
                                )
                        for tt in range(6):
                            nc.scalar.copy(out=v_m[:, tt, :], in_=accs[tt][:])
                        rest = range(6, BPC)
                    else:
                        qacc = None
                        rest = range(BPC)
                    for tt in rest:
                        acc = ps512.tile([128, 512], F32, tag="ps512",
                                         name="vacc")
                        for k in range(KC):
                            nc.tensor.matmul(
                                acc[:],
                                xt[k][:, tt * 128:(tt + 1) * 128],
                                wvs[:, k, :],
                                start=(k == 0), stop=(k == KC - 1),
                            )
                        nc.scalar.copy(out=v_m[:, tt, :], in_=acc[:])
                    return qacc

                def emit_qkproj(wc, cos_t, sin_t, dsth, accs=None):
                    if accs is None:
                        accs = [
                            ps512.tile([128, 512], F32, tag="ps512",
                                       name=f"qkacc{th}")
                            for th in range(TH)
                        ]
                        for k in range(KC):
                            for th in range(TH):
                                nc.tensor.matmul(
                                    accs[th][:],
                                    wc[:, k, :],
                                    xt[k][:, th * 512:(th + 1) * 512],
                                    start=(k == 0), stop=(k == KC - 1),
                                )
                    for th in range(TH):
                        ts_ = slice(th * 512, (th + 1) * 512)
                        qt_sb = stagep.tile([128, 512], BF16, tag="qt_sb")
                        nc.scalar.copy(out=qt_sb[:], in_=accs[th][:])
                        rot = mixp.tile([128, 512], F32, tag="mix", name="rot")
                        nc.tensor.matmul(rot[:], st_t[:], qt_sb[:],
                                         start=True, stop=True)
                        t1 = ropep.tile([128, 512], F32, tag="t1")
                        nc.vector.tensor_mul(t1[:], qt_sb[:], cos_t[:])
                        t2 = ropep.tile([128, 512], F32, tag="t2")
                        nc.vector.tensor_mul(t2[:], rot[:], sin_t[:])
                        nc.vector.tensor_add(dsth[:, ts_], t1[:], t2[:])

                def emit_attention(n, qh, kh, v_m, outh):
                    hs = slice((n % 4) * 128, (n % 4) * 128 + 128)
                    for b in range(BPC):
                        bs = slice(b * 128, (b + 1) * 128)
                        sc = psc.tile([128, 128], F32, tag="sc")
                        nc.tensor.matmul(sc[:], qh[:, bs], kh[:, bs],
                                         start=True, stop=True)
                        masked = softp.tile([128, 128], F32, tag="masked")
                        nc.vector.tensor_add(masked[:], sc[:], mask_t[:])
                        e_t = softp.tile([128, 128], BF16, tag="e")
                        sums = smallp.tile([128, 1], F32, tag="sums")
                        nc.scalar.activation(
                            out=e_t[:], in_=masked[:],
                            func=mybir.ActivationFunctionType.Exp,
                            scale=1.0, accum_out=sums[:],
                        )
                        inv = smallp.tile([128, 1], F32, tag="inv")
                        nc.vector.reciprocal(out=inv[:], in_=sums[:])
                        probs = softp.tile([128, 128], BF16, tag="probs")
                        nc.vector.tensor_scalar_mul(probs[:], e_t[:], inv[:])
                        pT = mixp.tile([128, 128], BF16, tag="mix", name="pT")
                        nc.tensor.transpose(pT[:], probs[:], id_t[:])
                        pT_sb = softp.tile([128, 128], BF16, tag="pT_sb")
                        nc.scalar.copy(out=pT_sb[:], in_=pT[:])
                        pv = mixp.tile([128, 128], F32, tag="mix", name="pv")
                        nc.tensor.matmul(pv[:], v_m[:, b, hs], pT_sb[:],
                                         start=True, stop=True)
                        nc.vector.tensor_copy(outh[:, bs], pv[:])

                for m in range(4):
                    if m == 0:
                        wvs = wvs0
                    else:
                        wvs = v_wvs_next
                    p0qacc = emit_vpass(m, wvs, interleave=(m == 0))

                    for n in range(4 * m, 4 * m + 4):
                        # prefetch next head's weights (2 heads of slack in
                        # the 4-buf wcol pool)
                        if n + 1 < H:
                            eng = nc.sync if n % 2 == 0 else nc.gpsimd
                            if (n + 1, "q") not in wcol:
                                _wcol_dma(n + 1, "q", eng)
                                _wcol_dma(n + 1, "k", eng)
                        # prefetch next V m-set / first wo m-sets
                        if n == 4 * m and m < 3:
                            v_wvs_next = wsp.tile([128, KC, 512], BF16,
                                                  tag="ws", name=f"wvs{m+1}")
                            for g in range(4):
                                nc.scalar.dma_start(
                                    out=v_wvs_next[:, 4 * g:4 * g + 4, :],
                                    in_=wv4[m + 1, :, 4 * g:4 * g + 4, :])
                        if n == 9 or n == 12:
                            wos = wsp.tile([128, KC, 512], BF16, tag="ws",
                                           name=f"wos{(n == 12) * 1}")
                            for g in range(4):
                                nc.scalar.dma_start(
                                    out=wos[:, 4 * g:4 * g + 4, :],
                                    in_=wo4[(n == 12) * 1, :, 4 * g:4 * g + 4, :])
                            if n == 9:
                                wos_tiles = [wos]
                            else:
                                wos_tiles.append(wos)

                        qh = qkhp.tile([128, TOK], BF16, tag="qh")
                        kh = qkhp.tile([128, TOK], BF16, tag="kh")
                        emit_qkproj(wcol.pop((n, "q")), cosq_t, sinq_t, qh,
                                    accs=(p0qacc if n == 0 else None))
                        emit_qkproj(wcol.pop((n, "k")), cosk_t, sink_t, kh)
                        emit_attention(n, qh, kh, v[m], attn[n])

            # ---- phase 3: y = attnT.T @ woT, all operands SBUF-resident ----
            with tc.tile_pool(name="ystage", bufs=3) as ystagep:
                for m in range(4):
                    if m + 2 < 4:
                        wos = wsp.tile([128, KC, 512], BF16, tag="ws",
                                       name=f"wos{m+2}")
                        for g in range(4):
                            nc.gpsimd.dma_start(
                                out=wos[:, 4 * g:4 * g + 4, :],
                                in_=wo4[m + 2, :, 4 * g:4 * g + 4, :])
                        wos_tiles.append(wos)
                    for tt in range(BPC):
                        acc = ps512.tile([128, 512], F32, tag="ps512",
                                         name="yacc")
                        for k in range(KC):
                            nc.tensor.matmul(
                                acc[:],
                                attn[k][:, tt * 128:(tt + 1) * 128],
                                wos_tiles[m][:, k, :],
                                start=(k == 0), stop=(k == KC - 1),
                            )
                        y_sb = ystagep.tile([128, 512], F32, tag="y_sb")
                        nc.scalar.copy(out=y_sb[:], in_=acc[:])
                        eng = nc.sync if tt % 2 == 0 else nc.gpsimd
                        ys = slice(tt * 128, (tt + 1) * 128)
                        if m == 3 and tt == BPC - 1:
                            # final tile: halve across both rings to shorten
                            # the post-compute drain
                            nc.sync.dma_start(
                                out=y[ys, m * 512:m * 512 + 256],
                                in_=y_sb[:, 0:256])
                            nc.gpsimd.dma_start(
                                out=y[ys, m * 512 + 256:(m + 1) * 512],
                                in_=y_sb[:, 256:512])
                        else:
                            eng.dma_start(
                                out=y[ys, m * 512:(m + 1) * 512],
                                in_=y_sb[:],
                            )

    nc.compile()
    return nc


def _prep_inputs(x, freqs_cos, freqs_sin, wq, wk, wv, wo):
    x = np.asarray(x, dtype=np.float32)
    fc = np.asarray(freqs_cos, dtype=np.float32)
    fs = np.asarray(freqs_sin, dtype=np.float32)
    wq = np.asarray(wq, np.float32)
    wk = np.asarray(wk, np.float32)
    wv = np.asarray(wv, np.float32)
    wo = np.asarray(wo, np.float32)
    shared = {
        # [n/m, p, kc, nn] = w[block*bw + nn, kc*128 + p]
        "wq4": np.ascontiguousarray(
            wq.reshape(H, 128, KC, 128).transpose(0, 3, 2, 1)).astype(BF),
        "wk4": np.ascontiguousarray(
            wk.reshape(H, 128, KC, 128).transpose(0, 3, 2, 1)).astype(BF),
        "wv4": np.ascontiguousarray(
            wv.reshape(4, 512, KC, 128).transpose(0, 3, 2, 1)).astype(BF),
        "wo4": np.ascontiguousarray(
            wo.reshape(4, 512, KC, 128).transpose(0, 3, 2, 1)).astype(BF),
    }
    st = np.zeros((128, 128), np.float32)
    for j in range(64):
        st[2 * j + 1, 2 * j] = -1.0
        st[2 * j, 2 * j + 1] = 1.0
    shared["stid"] = np.ascontiguousarray(
        np.concatenate([st, np.eye(128, dtype=np.float32)], axis=1)).astype(BF)
    shared["maskd"] = np.triu(np.full((128, 128), -1e30, np.float32), k=1)

    cosd = np.repeat(fc.T, 2, axis=0)  # [128, 128]: row d -> cos[t, d//2]
    sind = np.repeat(fs.T, 2, axis=0)
    cos4 = np.tile(cosd, (1, 4))  # [128, 512]
    sin4 = np.tile(sind, (1, 4))
    scale = np.float32(1.0 / np.sqrt(HD))
    shared["ropec"] = np.ascontiguousarray(
        np.concatenate([cos4 * scale, sin4 * scale, cos4, sin4],
                       axis=1)).astype(BF)

    in_maps = []
    for i in range(N_CORES):
        shard = x[i * BPC:(i + 1) * BPC].reshape(TOK, C)
        m = dict(shared)
        # [p, k, t] = shard.T[k*128+p, t]
        m["xT3"] = np.ascontiguousarray(
            shard.T.reshape(KC, 128, TOK).transpose(1, 0, 2)).astype(BF)
        in_maps.append(m)
    return in_maps


def _run(inputs, trace=False):
    if "nc" not in _CACHE:
        _CACHE["nc"] = _build()
    nc = _CACHE["nc"]
    in_maps = _prep_inputs(**inputs)
    res = run_bass_kernel_spmd(
        nc, in_maps, core_ids=list(range(N_CORES)), trace=trace
    )
    out = np.empty((B, T, C), np.float32)
    for i in range(N_CORES):
        out[i * BPC:(i + 1) * BPC] = np.asarray(res.results[i]["y"]).reshape(
            BPC, T, C
        )
    return out, res


def kernel(**inputs):
    out, _ = _run(inputs, trace=False)
    return out


# revision 49
# speedup vs baseline: 1.1948x; 1.1948x over previous
"""Trainium2 Bass kernel for nn_Attention_13297218748956.

Multi-head causal self-attention with RoPE (B=64, T=128, C=2048, H=16, hd=128),
data-parallel over batch across 8 NeuronCores (8 batches/core, no collectives).

Design (vs f32r baseline at ~667 us; this version ~514 us at full clock):
  - every matmul operand bf16 (1 cycle/row, LDWEIGHTS hidden in the
    background weight buffer, unlike f32r); fp32 PSUM accumulation.
    Validated numerically: rel err ~4e-3 vs the 2e-2 gate.
  - weights + xT shipped bf16 from host (halves DMA), V and per-head
    attention outputs stay SBUF-resident (no DRAM staging roundtrips).
  - attention math bf16: scores/transpose/probs@V are 128-cycle matmuls
    instead of fp32's 512.
  - softmax without max-subtraction (scores bounded ~|4.5|): exp+accum on
    ACT, one DVE mask-add, one DVE scale; rope tmps on DVE (gpsimd has no
    PSUM port and is slow).
  - prologue emitted k-outer over 6 V accs + head-0 q accs so the PE paces
    with the chunk DMA arrivals instead of stalling on whole tensors; a
    short dummy-matmul warm-up absorbs the cold p-state ramp during the
    first transfers' latency.
  - DMA descriptors each run on one hw queue (~45 GB/s): weight-stream
    prefetches (wvs/wos, 1 m-set ahead) are split 4-way for parallel
    queues; per-head wq/wk prefetched 1 head ahead on alternating rings;
    wos shares the wvs pool (wvs dead by the time wo streams in).
  - matmul operands always single-sliced 2D-tile APs (double-sliced 3D
    APs measured ~+40ns per matmul).

Per-core layout:
  xT3 [128, KC, 1024] bf16 chunk-major (1024 = 8 batches x 128 tokens)
  wq4/wk4 [H,128,KC,128] bf16: head col-blocks, partitions = contraction
  wv4/wo4 [4,128,KC,512] bf16: 512-wide m-set blocks
  phase 1 (per m): V m-set -> v[m] SBUF [128, 8, 512] bf16 (partition=token)
  phase 2 (per head): QT/KT = wcol.T @ xT -> rope in [d,t] layout (pair-swap
    matmul + cos/sin elementwise, 1/sqrt(hd) folded into q's cos/sin) ->
    per batch: scores -> mask-add -> exp -> scale -> PE-transpose ->
    attnT[n][:, b] = V.T @ probsT, attnT resident [128,1024] bf16 x16.
  phase 3: y = attnT.T @ woT streaming from SBUF, fp32 out.
"""

import numpy as np
import ml_dtypes

import concourse.bacc as bacc
import concourse.tile as tile
import concourse.mybir as mybir
from concourse.bass_utils import run_bass_kernel_spmd

N_CORES = 8
B, T, C, H = 64, 128, 2048, 16
HD = C // H  # 128
BPC = B // N_CORES  # 8 batches per core
TOK = BPC * T  # 1024 tokens per core
KC = C // 128  # 16 contraction chunks
TH = TOK // 512  # 2 token halves for 512-wide moving operand
F32 = mybir.dt.float32
BF16 = mybir.dt.bfloat16
BF = ml_dtypes.bfloat16

_CACHE = {}


def _build():
    nc = bacc.Bacc("TRN2", target_bir_lowering=False, debug=False)

    # xT3 is chunk-major: [p, k, t] = xT[k*128+p, t], so one SBUF tile can
    # receive the whole tensor in a few large descriptors
    xT3 = nc.dram_tensor("xT3", [128, KC, TOK], BF16, kind="ExternalInput")
    wq4 = nc.dram_tensor("wq4", [H, 128, KC, 128], BF16, kind="ExternalInput")
    wk4 = nc.dram_tensor("wk4", [H, 128, KC, 128], BF16, kind="ExternalInput")
    wv4 = nc.dram_tensor("wv4", [4, 128, KC, 512], BF16, kind="ExternalInput")
    wo4 = nc.dram_tensor("wo4", [4, 128, KC, 512], BF16, kind="ExternalInput")
    # stid = st | id packed; ropec = cosq|sinq|cosk|sink packed
    stid = nc.dram_tensor("stid", [128, 256], BF16, kind="ExternalInput")
    maskd = nc.dram_tensor("maskd", [128, 128], F32, kind="ExternalInput")
    ropec = nc.dram_tensor("ropec", [128, 2048], BF16, kind="ExternalInput")
    y = nc.dram_tensor("y", [TOK, C], F32, kind="ExternalOutput")

    with tile.TileContext(nc) as tc:
        with (
            tc.tile_pool(name="consts", bufs=1) as consts,
            tc.tile_pool(name="xt", bufs=1) as xtp,
            tc.tile_pool(name="vres", bufs=2) as vp,
            tc.tile_pool(name="attnres", bufs=1) as attnp,
            tc.tile_pool(name="wstream", bufs=2) as wsp,
            tc.tile_pool(name="wcol", bufs=4) as wcolp,
            tc.tile_pool(name="ps512", bufs=4, space="PSUM") as ps512,
            tc.tile_pool(name="psc", bufs=2, space="PSUM") as psc,
            tc.tile_pool(name="mix", bufs=2, space="PSUM") as mixp,
        ):
            stid_t = consts.tile([128, 256], BF16)
            st_t = stid_t[:, 0:128]
            id_t = stid_t[:, 128:256]
            mask_t = consts.tile([128, 128], F32)
            ropec_t = consts.tile([128, 2048], BF16)
            cosq_t = ropec_t[:, 0:512]
            sinq_t = ropec_t[:, 512:1024]
            cosk_t = ropec_t[:, 1024:1536]
            sink_t = ropec_t[:, 1536:2048]

            # ---- prologue DMAs. Descriptors each run on a single hw queue
            # (~45 GB/s), so bandwidth needs MANY in flight; 2D tiles keep
            # the matmul operand APs cheap (3D-sliced operands cost ~+40ns
            # per matmul, measured) ----
            # PE warm-up: a short dummy matmul stream with no DMA deps runs
            # during the first transfers' latency, ramping the clock out of
            # the cold p-state before real work arrives
            warm = consts.tile([128, 512], BF16, name="warm")
            nc.vector.memset(warm[:], 0.0)
            wacc = ps512.tile([128, 512], F32, tag="ps512", name="wacc")
            for _ in range(8):
                nc.tensor.matmul(wacc[:], warm[:, 0:128], warm[:],
                                 start=True, stop=True)

            xt = []
            for k in range(KC):
                xk = xtp.tile([128, TOK], BF16, tag=f"xt{k}", name=f"xt{k}")
                xt.append(xk)
            # sync: xt chunks in k order, first split for a fast start
            nc.sync.dma_start(out=xt[0][:, 0:128], in_=xT3[:, 0, 0:128])
            nc.sync.dma_start(out=xt[0][:, 128:256], in_=xT3[:, 0, 128:256])
            for q in range(1, 4):
                nc.sync.dma_start(out=xt[0][:, q * 256:(q + 1) * 256],
                                  in_=xT3[:, 0, q * 256:(q + 1) * 256])
            nc.sync.dma_start(out=xt[1][:, 0:512], in_=xT3[:, 1, 0:512])
            nc.sync.dma_start(out=xt[1][:, 512:1024], in_=xT3[:, 1, 512:1024])
            for k in range(2, KC - 2):
                nc.sync.dma_start(out=xt[k][:], in_=xT3[:, k, :])

            # scalar: wv m-set 0 per-k (paces the prologue V-pass), packed
            # consts interleaved by need time
            wvs0 = wsp.tile([128, KC, 512], BF16, tag="ws", name="wvs0")
            for k in range(KC):
                if k == 0:
                    # halves so the split first matmul starts sooner
                    nc.scalar.dma_start(out=wvs0[:, 0, 0:256],
                                        in_=wv4[0, :, 0, 0:256])
                    nc.scalar.dma_start(out=wvs0[:, 0, 256:512],
                                        in_=wv4[0, :, 0, 256:512])
                else:
                    nc.scalar.dma_start(out=wvs0[:, k, :], in_=wv4[0, :, k, :])
                if k == 6:
                    nc.scalar.dma_start(out=stid_t[:], in_=stid[:])
                elif k == 8:
                    nc.scalar.dma_start(out=ropec_t[:], in_=ropec[:])
                elif k == 10:
                    nc.scalar.dma_start(out=mask_t[:], in_=maskd[:])
            # head 0/1 weights in 4-chunk groups (subtile deps let proj start
            # before the full 0.5MB lands)
            wcol = {}

            def _wcol_dma(n, kind, eng, groups=1):
                wT_ = wq4 if kind == "q" else wk4
                wc = wcolp.tile([128, KC, 128], BF16, tag="wcol",
                                name=f"wcol_{kind}{n}")
                gsz = KC // groups
                for g in range(groups):
                    eng.dma_start(out=wc[:, g * gsz:(g + 1) * gsz, :],
                                  in_=wT_[n, :, g * gsz:(g + 1) * gsz, :])
                wcol[(n, kind)] = wc
            # wq0 first and finely split: the prologue q-interleave matmuls
            # sit in the PE queue, so a late wq0 chunk stalls everything.
            # gpsimd carries ONLY the head-0/1 weights: a late wq1 caused a
            # 4us PE gap + HAM re-throttle
            _wcol_dma(0, "q", nc.gpsimd, groups=4)
            # two late xt chunks ride gpsimd's less-congested issue queue
            for k in range(KC - 2, KC):
                nc.gpsimd.dma_start(out=xt[k][:], in_=xT3[:, k, :])
            _wcol_dma(0, "k", nc.gpsimd, groups=2)
            _wcol_dma(1, "q", nc.gpsimd, groups=2)
            _wcol_dma(1, "k", nc.gpsimd, groups=2)

            v = {}
            attn = []
            for n in range(H):
                an = attnp.tile([128, TOK], BF16, tag=f"at{n}", name=f"at{n}")
                attn.append(an)

            with (
                tc.tile_pool(name="qkh", bufs=2) as qkhp,
                tc.tile_pool(name="qtstage", bufs=2) as stagep,
                tc.tile_pool(name="ropet", bufs=2) as ropep,
                tc.tile_pool(name="soft", bufs=3) as softp,
                tc.tile_pool(name="small", bufs=4) as smallp,
            ):

                def emit_vpass(m, wvs, interleave):
                    v_m = vp.tile([128, BPC, 512], BF16, tag="v", name=f"v{m}")
                    v[m] = v_m
                    if interleave:
                        # k-outer over 6 V accs + head-0 q accs (borrowing
                        # the psc slots, which grow to bank size): 8 matmuls
                        # per chunk paces the PE with the chunk DMA arrivals
                        accs = [
                            ps512.tile([128, 512], F32, tag="ps512",
                                       name=f"vacc{tt}")
                            for tt in range(4)
                        ] + [
                            mixp.tile([128, 512], F32, tag="mix",
                                      name=f"vacc{tt}")
                            for tt in range(4, 6)
                        ]
                        qacc = [
                            psc.tile([128, 512], F32, tag="sc",
                                     name=f"p0qacc{th}")
                            for th in range(TH)
                        ]
                        wq0 = wcol[(0, "q")]
                        for k in range(KC):
                            for tt in range(6):
                                nc.tensor.matmul(
                                    accs[tt][:],
                                    xt[k][:, tt * 128:(tt + 1) * 128],
                                    wvs[:, k, :],
                                    start=(k == 0), stop=(k == KC - 1),
                                )
                            for th in range(TH):
                                nc.tensor.matmul(
                                    qacc[th][:],
                                    wq0[:, k, :],
                                    xt[k][:, th * 512:(th + 1) * 512],
                                    start=(k == 0), stop=(k == KC - 1),
                                )
                        for tt in range(6):
                            nc.scalar.copy(out=v_m[:, tt, :], in_=accs[tt][:])
                        rest = range(6, BPC)
                    else:
                        qacc = None
                        rest = range(BPC)
                    for tt in rest:
                        acc = ps512.tile([128, 512], F32, tag="ps512",
                                         name="vacc")
                        for k in range(KC):
                            nc.tensor.matmul(
                                acc[:],
                                xt[k][:, tt * 128:(tt + 1) * 128],
                                wvs[:, k, :],
                                start=(k == 0), stop=(k == KC - 1),
                            )
                        nc.scalar.copy(out=v_m[:, tt, :], in_=acc[:])
                    return qacc

                def emit_qkproj(wc, cos_t, sin_t, dsth, accs=None):
                    if accs is None:
                        accs = [
                            ps512.tile([128, 512], F32, tag="ps512",
                                       name=f"qkacc{th}")
                            for th in range(TH)
                        ]
                        for k in range(KC):
                            for th in range(TH):
                                nc.tensor.matmul(
                                    accs[th][:],
                                    wc[:, k, :],
                                    xt[k][:, th * 512:(th + 1) * 512],
                                    start=(k == 0), stop=(k == KC - 1),
                                )
                    for th in range(TH):
                        ts_ = slice(th * 512, (th + 1) * 512)
                        qt_sb = stagep.tile([128, 512], BF16, tag="qt_sb")
                        nc.scalar.copy(out=qt_sb[:], in_=accs[th][:])
                        rot = mixp.tile([128, 512], F32, tag="mix", name="rot")
                        nc.tensor.matmul(rot[:], st_t[:], qt_sb[:],
                                         start=True, stop=True)
                        t1 = ropep.tile([128, 512], F32, tag="t1")
                        nc.vector.tensor_mul(t1[:], qt_sb[:], cos_t[:])
                        t2 = ropep.tile([128, 512], F32, tag="t2")
                        nc.vector.tensor_mul(t2[:], rot[:], sin_t[:])
                        nc.vector.tensor_add(dsth[:, ts_], t1[:], t2[:])

                def emit_attention(n, qh, kh, v_m, outh):
                    hs = slice((n % 4) * 128, (n % 4) * 128 + 128)
                    for b in range(BPC):
                        bs = slice(b * 128, (b + 1) * 128)
                        sc = psc.tile([128, 128], F32, tag="sc")
                        nc.tensor.matmul(sc[:], qh[:, bs], kh[:, bs],
                                         start=True, stop=True)
                        masked = softp.tile([128, 128], F32, tag="masked")
                        nc.vector.tensor_add(masked[:], sc[:], mask_t[:])
                        e_t = softp.tile([128, 128], BF16, tag="e")
                        sums = smallp.tile([128, 1], F32, tag="sums")
                        nc.scalar.activation(
                            out=e_t[:], in_=masked[:],
                            func=mybir.ActivationFunctionType.Exp,
                            scale=1.0, accum_out=sums[:],
                        )
                        inv = smallp.tile([128, 1], F32, tag="inv")
                        nc.vector.reciprocal(out=inv[:], in_=sums[:])
                        probs = softp.tile([128, 128], BF16, tag="probs")
                        nc.vector.tensor_scalar_mul(probs[:], e_t[:], inv[:])
                        pT = mixp.tile([128, 128], BF16, tag="mix", name="pT")
                        nc.tensor.transpose(pT[:], probs[:], id_t[:])
                        pT_sb = softp.tile([128, 128], BF16, tag="pT_sb")
                        nc.scalar.copy(out=pT_sb[:], in_=pT[:])
                        pv = mixp.tile([128, 128], F32, tag="mix", name="pv")
                        nc.tensor.matmul(pv[:], v_m[:, b, hs], pT_sb[:],
                                         start=True, stop=True)
                        nc.vector.tensor_copy(outh[:, bs], pv[:])

                for m in range(4):
                    if m == 0:
                        wvs = wvs0
                    else:
                        wvs = v_wvs_next
                    p0qacc = emit_vpass(m, wvs, interleave=(m == 0))

                    for n in range(4 * m, 4 * m + 4):
                        # prefetch next head's weights (2 heads of slack in
                        # the 4-buf wcol pool)
                        if n + 1 < H:
                            eng = nc.sync if n % 2 == 0 else nc.gpsimd
                            if (n + 1, "q") not in wcol:
                                _wcol_dma(n + 1, "q", eng)
                                _wcol_dma(n + 1, "k", eng)
                        # prefetch next V m-set / first wo m-sets
                        if n == 4 * m and m < 3:
                            v_wvs_next = wsp.tile([128, KC, 512], BF16,
                                                  tag="ws", name=f"wvs{m+1}")
                            for g in range(4):
                                nc.scalar.dma_start(
                                    out=v_wvs_next[:, 4 * g:4 * g + 4, :],
                                    in_=wv4[m + 1, :, 4 * g:4 * g + 4, :])
                        if n == 9 or n == 12:
                            wos = wsp.tile([128, KC, 512], BF16, tag="ws",
                                           name=f"wos{(n == 12) * 1}")
                            for g in range(4):
                                nc.scalar.dma_start(
                                    out=wos[:, 4 * g:4 * g + 4, :],
                                    in_=wo4[(n == 12) * 1, :, 4 * g:4 * g + 4, :])
                            if n == 9:
                                wos_tiles = [wos]
                            else:
                                wos_tiles.append(wos)

                        qh = qkhp.tile([128, TOK], BF16, tag="qh")
                        kh = qkhp.tile([128, TOK], BF16, tag="kh")
                        emit_qkproj(wcol.pop((n, "q")), cosq_t, sinq_t, qh,
                                    accs=(p0qacc if n == 0 else None))
                        emit_qkproj(wcol.pop((n, "k")), cosk_t, sink_t, kh)
                        emit_attention(n, qh, kh, v[m], attn[n])

            # ---- phase 3: y = attnT.T @ woT, all operands SBUF-resident ----
            with tc.tile_pool(name="ystage", bufs=3) as ystagep:
                for m in range(4):
                    if m + 2 < 4:
                        wos = wsp.tile([128, KC, 512], BF16, tag="ws",
                                       name=f"wos{m+2}")
                        for g in range(4):
                            nc.gpsimd.dma_start(
                                out=wos[:, 4 * g:4 * g + 4, :],
                                in_=wo4[m + 2, :, 4 * g:4 * g + 4, :])
                        wos_tiles.append(wos)
                    for tt in range(BPC):
                        acc = ps512.tile([128, 512], F32, tag="ps512",
                                         name="yacc")
                        for k in range(KC):
                            nc.tensor.matmul(
                                acc[:],
                                attn[k][:, tt * 128:(tt + 1) * 128],
                                wos_tiles[m][:, k, :],
                                start=(k == 0), stop=(k == KC - 1),
                            )
                        y_sb = ystagep.tile([128, 512], F32, tag="y_sb")
                        nc.scalar.copy(out=y_sb[:], in_=acc[:])
                        eng = nc.sync if tt % 2 == 0 else nc.gpsimd
                        ys = slice(tt * 128, (tt + 1) * 128)
                        if m == 3 and tt >= BPC - 2:
                            # final tiles: quarter across both rings to
                            # shorten the post-compute drain
                            for q in range(4):
                                qeng = nc.sync if q % 2 == 0 else nc.gpsimd
                                qs = slice(m * 512 + q * 128,
                                           m * 512 + (q + 1) * 128)
                                qeng.dma_start(out=y[ys, qs],
                                               in_=y_sb[:, q * 128:
                                                        (q + 1) * 128])
                        else:
                            eng.dma_start(
                                out=y[ys, m * 512:(m + 1) * 512],
                                in_=y_sb[:],
                            )

    nc.compile()
    return nc


def _prep_inputs(x, freqs_cos, freqs_sin, wq, wk, wv, wo):
    x = np.asarray(x, dtype=np.float32)
    fc = np.asarray(freqs_cos, dtype=np.float32)
    fs = np.asarray(freqs_sin, dtype=np.float32)
    wq = np.asarray(wq, np.float32)
    wk = np.asarray(wk, np.float32)
    wv = np.asarray(wv, np.float32)
    wo = np.asarray(wo, np.float32)
    shared = {
        # [n/m, p, kc, nn] = w[block*bw + nn, kc*128 + p]
        "wq4": np.ascontiguousarray(
            wq.reshape(H, 128, KC, 128).transpose(0, 3, 2, 1)).astype(BF),
        "wk4": np.ascontiguousarray(
            wk.reshape(H, 128, KC, 128).transpose(0, 3, 2, 1)).astype(BF),
        "wv4": np.ascontiguousarray(
            wv.reshape(4, 512, KC, 128).transpose(0, 3, 2, 1)).astype(BF),
        "wo4": np.ascontiguousarray(
            wo.reshape(4, 512, KC, 128).transpose(0, 3, 2, 1)).astype(BF),
    }
    st = np.zeros((128, 128), np.float32)
    for j in range(64):
        st[2 * j + 1, 2 * j] = -1.0
        st[2 * j, 2 * j + 1] = 1.0
    shared["stid"] = np.ascontiguousarray(
        np.concatenate([st, np.eye(128, dtype=np.float32)], axis=1)).astype(BF)
    shared["maskd"] = np.triu(np.full((128, 128), -1e30, np.float32), k=1)

    cosd = np.repeat(fc.T, 2, axis=0)  # [128, 128]: row d -> cos[t, d//2]
    sind = np.repeat(fs.T, 2, axis=0)
    cos4 = np.tile(cosd, (1, 4))  # [128, 512]
    sin4 = np.tile(sind, (1, 4))
    scale = np.float32(1.0 / np.sqrt(HD))
    shared["ropec"] = np.ascontiguousarray(
        np.concatenate([cos4 * scale, sin4 * scale, cos4, sin4],
                       axis=1)).astype(BF)

    in_maps = []
    for i in range(N_CORES):
        shard = x[i * BPC:(i + 1) * BPC].reshape(TOK, C)
        m = dict(shared)
        # [p, k, t] = shard.T[k*128+p, t]
        m["xT3"] = np.ascontiguousarray(
            shard.T.reshape(KC, 128, TOK).transpose(1, 0, 2)).astype(BF)
        in_maps.append(m)
    return in_maps


def _run(inputs, trace=False):
    if "nc" not in _CACHE:
        _CACHE["nc"] = _build()
    nc = _CACHE["nc"]
    in_maps = _prep_inputs(**inputs)
    res = run_bass_kernel_spmd(
        nc, in_maps, core_ids=list(range(N_CORES)), trace=trace
    )
    out = np.empty((B, T, C), np.float32)
    for i in range(N_CORES):
        out[i * BPC:(i + 1) * BPC] = np.asarray(res.results[i]["y"]).reshape(
            BPC, T, C
        )
    return out, res


def kernel(**inputs):
    out, _ = _run(inputs, trace=False)
    return out
